# revision 14
# baseline (speedup 1.0000x reference)
"""AdapterGNN distributed Trainium2 kernel (8 NeuronCores, Bass/Tile).

out = norm_dst * segsum_dst( ((X*norm_src) @ Wd + norm_src*bd)[src] ) @ (Wg@Wu) + (bg@Wu+bu)

Sharding: nodes split across 8 cores; edges partitioned by dst owner. The
down-projected node features h (fp16) are AllGathered into a DRAM table so
per-edge h[src] reads are core-local.

Aggregation: edges are gathered DENSELY (no per-dst rectangular padding) with
batched dma_gather instructions — one per (window-group, table-region) — into
an SBUF stream G laid out [edge%128 (partition), edge//128 (block), feat].
The int16 gather index limits one instruction to a 32704-row table region, so
the table is covered by 4 overlapping regions and each edge is routed to the
region (= table quarter) containing its src row. The segment-sum is computed
on the PE: for each 128-edge block, a selection matrix S[e, d] =
is_equal(iota, slot_e) * norm_dst_e is built by one DVE tensor_scalar op and
psum[f, d] += G_blk^T-contract-S accumulates aggT per window directly (no
transpose needed). Per-(window, region) edge counts are shared across cores
(max + pad with slot=-1 sentinels) so the block -> psum-slice mapping is
identical on every core. The up-projection uses the fused (Wg@Wu) weight; the
bias rides a K=1 matmul; psum drains to fp16 on the Activation engine.

Self-contained: requires only numpy + concourse (+ TRN2 cores via axon).
"""

import os

import numpy as np

import concourse.bacc as bacc
import concourse.mybir as mybir
import concourse.tile as tile
from concourse.bass_utils import run_bass_kernel_spmd

F32 = mybir.dt.float32
F16 = mybir.dt.float16
I16 = mybir.dt.int16

P = 128  # partitions

RLEN = 32704          # rows addressable by one int16-indexed gather
WPG = 6               # windows per group (psum budget: (WPG+1)*128 f32 = 2 banks)
MAXH = 4              # max window-halves one block may span
IOTA_W = MAXH * P     # iota width in consts
ONES_O = IOTA_W       # ones column block offset in consts
BU2_O = IOTA_W + P    # bias row offset in consts


class Cfg:
    def __init__(self, n_nodes, n_edges, in_dim, out_dim, n_cores=8):
        self.N = n_nodes
        self.E = n_edges
        self.IN = in_dim          # 768
        self.OUT = out_dim        # 128 (must be 128)
        self.C = n_cores
        assert out_dim == P
        self.NpReal = (n_nodes + n_cores - 1) // n_cores   # real nodes per core
        self.W = (self.NpReal + P - 1) // P                # windows per core
        self.Np = self.W * P                               # padded nodes/core
        self.KC = in_dim // P                              # full K chunks (6)
        assert in_dim % P == 0
        self.KIN = self.KC + 1                             # +1 chunk for (norm,bias) row
        self.n_rows = n_cores * self.Np


def _ceil128(x):
    return int(-(-int(x) // 128) * 128)


def host_prep(cfg, features, Wd, bd, Wg, bg, Wu, bu, src, dst):
    """Returns (in_maps, node_core, node_slot, prof)."""
    C, N, Np, W = cfg.C, cfg.N, cfg.Np, cfg.W
    src = np.asarray(src).astype(np.int64)
    dst = np.asarray(dst).astype(np.int64)
    features = np.asarray(features, dtype=np.float32)

    out_deg = np.bincount(src, minlength=N)
    in_deg = np.bincount(dst, minlength=N)
    norm_src = 1.0 / np.sqrt(np.maximum(out_deg, 1.0))
    norm_dst = 1.0 / np.sqrt(np.maximum(in_deg, 1.0))

    # node -> core; slot = in-degree order (keeps per-window edge counts
    # similar across cores -> tight shared maxima). The h table row equals the
    # slot the down-projection writes to, so table_row is slot-keyed.
    node_core = np.minimum(np.arange(N) // cfg.NpReal, C - 1)
    node_slot = np.empty(N, dtype=np.int64)
    for c in range(C):
        ns = np.where(node_core == c)[0]
        order = np.argsort(-in_deg[ns], kind="stable")
        node_slot[ns[order]] = np.arange(len(ns))
    table_row = node_core * Np + node_slot

    # regions 0-3: overlapping 32704-row windows of the REMOTE table (edges
    # whose src lives on another core), flex-balanced between adjacent regions
    # in the overlap zones so per-(window, region) counts are near-equal.
    # region 4: OWN edges (src on dst's core), gathered from local h_mine
    # (idx = local slot) concurrently with the AllGather.
    n_rows = cfg.n_rows
    rlen = min(RLEN, n_rows)
    nbase = n_rows - rlen
    bases = [0, nbase // 3, (2 * nbase) // 3, nbase]

    erow = table_row[src]
    ecore = node_core[dst]
    eslot = node_slot[dst]
    ew = eslot // P
    ep = eslot % P
    own = node_core[src] == ecore

    # region whose base is the highest <= erow; flexible iff also inside the
    # previous region's window
    zb = np.searchsorted(np.array(bases[1:]), erow, side="right")
    basesa = np.array(bases)
    flex = (zb >= 1) & (erow < basesa[np.maximum(zb - 1, 0)] + rlen) & ~own
    key = ecore * W + ew

    fcnt = np.zeros((C * W, 4), dtype=np.int64)
    np.add.at(fcnt, (key[~own & ~flex], zb[~own & ~flex]), 1)
    zcnt = np.zeros((C * W, 3), dtype=np.int64)
    np.add.at(zcnt, (key[flex], zb[flex] - 1), 1)

    tgt = (fcnt.sum(1) + zcnt.sum(1) + 3) // 4
    a = np.zeros((C * W, 3), dtype=np.int64)
    run = fcnt[:, 0]
    a[:, 0] = np.clip(tgt - run, 0, zcnt[:, 0])
    run = fcnt[:, 1] + (zcnt[:, 0] - a[:, 0])
    a[:, 1] = np.clip(tgt - run, 0, zcnt[:, 1])
    run = fcnt[:, 2] + (zcnt[:, 1] - a[:, 1])
    a[:, 2] = np.clip(tgt - run, 0, zcnt[:, 2])

    # per-edge final region
    ereg = np.where(own, 4, zb)
    fi = np.where(flex)[0]
    fz = zb[fi] - 1
    order = np.lexsort((erow[fi], fz, key[fi]))
    fi, fz = fi[order], fz[order]
    segid = key[fi] * 3 + fz
    uniq, counts = np.unique(segid, return_counts=True)
    cum = np.concatenate([[0], np.cumsum(counts)])
    frank = np.arange(len(fi)) - cum[np.searchsorted(uniq, segid)]
    ereg[fi] = fz + (frank >= a[key[fi], fz])

    # per-(core, window, region) counts and shared maxima
    NC = np.zeros((C, W, 5), dtype=np.int64)
    np.add.at(NC, (ecore, ew, ereg), 1)
    M = NC.max(axis=0)  # [W, 5]

    # groups of windows
    groups = []
    w0 = 0
    while w0 < W:
        nw = min(WPG, W - w0)
        groups.append((w0, nw))
        w0 += nw
    NG = len(groups)

    # chunk layout: all OWN chunks first (they run during the AllGather),
    # then remote chunks per (group, region)
    chunk_off = np.zeros((NG, 5), dtype=np.int64)
    chunk_len = np.zeros((NG, 5), dtype=np.int64)
    blocks = []  # (g, r, k_in_chunk, lo_seg, n_halves, bidx_global)
    off = 0

    def add_chunk(g, r, gw0, nw):
        nonlocal off
        seg = M[gw0 : gw0 + nw, r]
        L = _ceil128(seg.sum())
        chunk_off[g, r] = off
        chunk_len[g, r] = L
        bcum = np.concatenate([[0], np.cumsum(seg)])
        for k in range(L // 128):
            p0, p1 = k * 128, k * 128 + 127
            lo = int(np.searchsorted(bcum[1:], p0, side="right"))
            hi = int(np.searchsorted(bcum[1:], p1, side="right"))
            lo, hi = min(lo, nw), min(hi, nw)
            nh = hi - lo + 1
            assert nh <= MAXH, f"block spans {nh} windows"
            blocks.append([g, r, k, lo, nh, len(blocks)])
        off += L

    for g, (gw0, nw) in enumerate(groups):
        add_chunk(g, 4, gw0, nw)
    for g, (gw0, nw) in enumerate(groups):
        for r in range(4):
            add_chunk(g, r, gw0, nw)
    T = int(off)            # total stream length
    NBLK = len(blocks)

    # start/stop flags per (own?, g, psum zero-region): a 2KB PSUM zero region
    # holds 4 window slices (512B each). start=True marks the WHOLE region
    # pending-zero (lazy zeroing), so exactly one start and one stop per
    # region, on its chronologically first/last matmul. Own chunks accumulate
    # into their own psum tiles, so they get a separate flag namespace.
    first_mm = {}
    last_mm = {}
    for b in blocks:
        g, r, k, lo, nh, bi = b
        for h in range(nh):
            key2 = (r == 4, g, (lo + h) // 4)
            if key2 not in first_mm:
                first_mm[key2] = (bi, h)
            last_mm[key2] = (bi, h)

    # shared profile for build_graph (hashable)
    prof = {
        "groups": tuple(groups),
        "chunk_off": tuple(map(tuple, chunk_off)),
        "chunk_len": tuple(map(tuple, chunk_len)),
        "blocks": tuple(tuple(b) for b in blocks),
        "first": frozenset((k, v) for k, v in first_mm.items()),
        "last": frozenset((k, v) for k, v in last_mm.items()),
        "T": T,
        "NBLK": NBLK,
        "bases": tuple(bases),
        "rlen": rlen,
    }

    # fused weights
    Wgu = (np.asarray(Wg, np.float64) @ np.asarray(Wu, np.float64)).astype(np.float32)
    bu2 = (np.asarray(bg, np.float64) @ np.asarray(Wu, np.float64) + bu).astype(np.float32)

    wd_h = np.zeros((P, cfg.KIN * cfg.OUT), dtype=np.float16)
    for cc in range(cfg.KC):
        wd_h[:, cc * cfg.OUT : (cc + 1) * cfg.OUT] = Wd[cc * P : (cc + 1) * P, :]
    wd_h[0, cfg.KC * cfg.OUT : (cfg.KC + 1) * cfg.OUT] = bd
    wgu_h = Wgu.astype(np.float16)

    consts = np.zeros((P, BU2_O + cfg.IN), dtype=np.float16)
    consts[:, 0:IOTA_W] = np.arange(IOTA_W, dtype=np.float16)[None, :]
    consts[:, ONES_O : ONES_O + P] = 1.0
    consts[:, BU2_O : BU2_O + cfg.IN] = bu2.astype(np.float16)[None, :]

    # per-core streams
    S16 = T // 16
    in_maps = []
    # intra-chunk window segment offsets (shared)
    segoff = np.zeros((W, 5), dtype=np.int64)
    for g, (gw0, nw) in enumerate(groups):
        for r in range(5):
            segoff[gw0 : gw0 + nw, r] = chunk_off[g, r] + np.concatenate(
                [[0], np.cumsum(M[gw0 : gw0 + nw, r])[:-1]]
            )

    blk_lo = np.zeros(T // 128, dtype=np.int64)
    for g, r, k, lo, nh, bi in blocks:
        assert chunk_off[g, r] // 128 + k == bi
        blk_lo[bi] = lo

    eslot_src = node_slot[src]  # own-region gather row (local h_mine slot)
    gw0_of = np.array([groups[gg][0] for gg in range(NG)])

    for c in range(C):
        em = np.where(ecore == c)[0]
        ewc, erc, epc = ew[em], ereg[em], ep[em]
        order = np.lexsort((epc, erc, ewc))
        em, ewc, erc, epc = em[order], ewc[order], erc[order], epc[order]
        segid = ewc * 5 + erc
        uniq, counts = np.unique(segid, return_counts=True)
        cum = np.concatenate([[0], np.cumsum(counts)])
        rank = np.arange(len(em)) - cum[np.searchsorted(uniq, segid)]
        pos = segoff[ewc, erc] + rank
        assert (rank < M[ewc, erc]).all()

        idx_s = np.zeros(T, dtype=np.int64)
        slotg = np.full(T, -1, dtype=np.int64)   # slot within group (wi*128+p)
        ndst_s = np.zeros(T, dtype=np.float32)
        idxval = np.where(erc == 4, eslot_src[em], erow[em] - basesa[np.minimum(erc, 3)])
        idx_s[pos] = idxval
        g_of_w = np.minimum(ewc // WPG, NG - 1)
        wi = ewc - gw0_of[g_of_w]
        slotg[pos] = wi * P + epc
        ndst_s[pos] = norm_dst[dst[em]].astype(np.float32)
        assert (idx_s >= 0).all() and (idx_s < max(rlen, Np)).all()

        # per-block relative slots
        slot_rel = slotg.reshape(-1, 128) - blk_lo[:, None] * P
        slot_rel[slotg.reshape(-1, 128) < 0] = -1
        assert (slot_rel < MAXH * P).all()

        idx16 = np.zeros((P, S16), dtype=np.int16)
        sidx = np.arange(S16) * 16
        for p in range(P):
            idx16[p, :] = idx_s[sidx + (p % 16)]
        slotv = np.ascontiguousarray(slot_rel.T.astype(np.float32))   # [128, NBLK]
        ndstv = np.ascontiguousarray(
            ndst_s.reshape(-1, 128).T.astype(np.float32)
        )

        # xa: window-blocked [p, w*KIN*128 + cc*128 + n]
        nt_ids = np.where(node_core == np.int64(c))[0]
        n_real = len(nt_ids)
        xs = (features[nt_ids, :] * norm_src[nt_ids, None]).astype(np.float16)
        xa = np.zeros((P, W * cfg.KIN * P), dtype=np.float16)
        xs_slot = np.zeros((Np, cfg.IN), dtype=np.float16)
        xs_slot[node_slot[nt_ids], :] = xs
        nsr = np.zeros(Np, dtype=np.float16)
        nsr[node_slot[nt_ids]] = norm_src[nt_ids].astype(np.float16)
        for w in range(W):
            blkb = w * cfg.KIN * P
            rows = xs_slot[w * P : (w + 1) * P, :]  # [128 nodes, IN]
            for cc in range(cfg.KC):
                xa[:, blkb + cc * P : blkb + (cc + 1) * P] = rows[:, cc * P : (cc + 1) * P].T
            xa[0, blkb + cfg.KC * P : blkb + (cfg.KC + 1) * P] = nsr[w * P : (w + 1) * P]

        in_maps.append(
            {
                "xa": xa,
                "idx": idx16,
                "slotv": slotv,
                "ndstv": ndstv,
                "wd": wd_h,
                "wgu": wgu_h,
                "consts": consts,
            }
        )

    return in_maps, node_core, node_slot, prof


def build_graph(cfg, prof):
    """Build the SPMD Bass graph (same for all cores)."""
    W = cfg.W
    OUT, IN = cfg.OUT, cfg.IN
    groups = prof["groups"]
    chunk_off = prof["chunk_off"]
    chunk_len = prof["chunk_len"]
    blocks = prof["blocks"]
    first_mm = dict(prof["first"])
    last_mm = dict(prof["last"])
    T = prof["T"]
    NBLK = prof["NBLK"]
    bases = prof["bases"]
    rlen = prof["rlen"]
    S16 = T // 16

    nc = bacc.Bacc(None, target_bir_lowering=False)
    xa = nc.declare_dram_parameter("xa", [P, W * cfg.KIN * P], F16, False)
    idx = nc.declare_dram_parameter("idx", [P, S16], I16, False)
    slotv = nc.declare_dram_parameter("slotv", [P, NBLK], F32, False)
    ndstv = nc.declare_dram_parameter("ndstv", [P, NBLK], F32, False)
    wd = nc.declare_dram_parameter("wd", [P, cfg.KIN * OUT], F16, False)
    wgu = nc.declare_dram_parameter("wgu", [OUT, IN], F16, False)
    consts = nc.declare_dram_parameter("consts", [P, BU2_O + IN], F16, False)
    out = nc.declare_dram_parameter("out", [cfg.Np, IN], F16, True)

    with tile.TileContext(nc) as tc:
        with tc.tile_pool(name="dram", bufs=1, space="DRAM") as dram:
            h_mine = dram.tile([cfg.Np, OUT], F16)
            h_all = dram.tile([cfg.n_rows, OUT], F16)

            # ---- phase A: down-projection ----
            with (
                tc.tile_pool(name="aconst", bufs=1) as aconst,
                tc.tile_pool(name="xat", bufs=2) as xap,
                tc.tile_pool(name="hst", bufs=1) as hstp,
                tc.tile_pool(name="dpsum", bufs=2, space="PSUM") as dpsum,
            ):
                wd_sb = aconst.tile([P, cfg.KIN * OUT], F16)
                nc.sync.dma_start(out=wd_sb[:], in_=wd[:, :])
                h_stage = hstp.tile([P, W * OUT], F16)
                xa_v = xa[:, :].rearrange("p (w x) -> p w x", w=W)
                NQ = 4
                wq = (W + NQ - 1) // NQ
                for qd in range(NQ):
                    lo = qd * wq
                    hi = min(lo + wq, W)
                    xt = xap.tile([P, (hi - lo) * cfg.KIN * P], F16)
                    nc.sync.dma_start(
                        out=xt[:].rearrange("p (w x) -> p w x", w=hi - lo),
                        in_=xa_v[:, lo:hi, :],
                    )
                    for w in range(lo, hi):
                        blkb = (w - lo) * cfg.KIN * P
                        ps = dpsum.tile([P, OUT], F32, space="PSUM")
                        for ccx in range(cfg.KC):
                            nc.tensor.matmul(
                                ps[:],
                                lhsT=xt[:, blkb + ccx * P : blkb + (ccx + 1) * P],
                                rhs=wd_sb[:, ccx * OUT : (ccx + 1) * OUT],
                                start=(ccx == 0),
                                stop=False,
                            )
                        nc.tensor.matmul(
                            ps[:],
                            lhsT=xt[0:1, blkb + cfg.KC * P : blkb + (cfg.KC + 1) * P],
                            rhs=wd_sb[0:1, cfg.KC * OUT : cfg.KC * OUT + OUT],
                            start=False,
                            stop=True,
                        )
                        nc.vector.tensor_copy(
                            out=h_stage[:, w * OUT : (w + 1) * OUT], in_=ps[:]
                        )
                # one DMA: h_stage [p, w, f] -> h_mine rows (w*128+p)
                nc.sync.dma_start(
                    out=h_mine[:, :].rearrange("(w p) f -> p w f", p=P),
                    in_=h_stage[:].rearrange("p (w f) -> p w f", w=W),
                )

            # ---- all-gather h ----
            if os.environ.get("GNN_NO_COLL"):
                # debug: fake the collective with a local copy (wrong numbers)
                for c in range(cfg.C):
                    nc.sync.dma_start(
                        out=h_all[c * cfg.Np : (c + 1) * cfg.Np, :], in_=h_mine[:, :]
                    )
            else:
                nc.gpsimd.collective_compute(
                    "AllGather",
                    mybir.AluOpType.bypass,
                    replica_groups=[list(range(cfg.C))],
                    ins=[h_mine[:].opt()],
                    outs=[h_all[:].opt()],
                )

            # ---- phase B: edge aggregation + up-projection ----
            with (
                tc.tile_pool(name="bconst", bufs=1) as bconst,
                tc.tile_pool(name="gp", bufs=3) as gp,
                tc.tile_pool(name="sp", bufs=8) as sp,
                tc.tile_pool(name="gpsum", bufs=1, space="PSUM") as gpsum,
                tc.tile_pool(name="upsum", bufs=2, space="PSUM") as upsum,
                tc.tile_pool(name="agp", bufs=2) as agp,
                tc.tile_pool(name="osb", bufs=2) as osb,
            ):
                wgu_sb = bconst.tile([OUT, IN], F16)
                nc.sync.dma_start(out=wgu_sb[:], in_=wgu[:, :])
                cst = bconst.tile([P, BU2_O + IN], F16)
                nc.sync.dma_start(out=cst[:], in_=consts[:, :])
                idx_sb = bconst.tile([P, S16], I16)
                nc.sync.dma_start(out=idx_sb[:], in_=idx[:, :])
                slot_sb = bconst.tile([P, NBLK], F32)
                nc.sync.dma_start(out=slot_sb[:], in_=slotv[:, :])
                ndst_sb = bconst.tile([P, NBLK], F32)
                nc.sync.dma_start(out=ndst_sb[:], in_=ndstv[:, :])
                ownAgg = bconst.tile([P, W * P], F16)

                def do_blocks(g, r, gt, ps, ownf):
                    for g2, r2, k, lo, nh, bi in blocks:
                        if g2 != g or r2 != r:
                            continue
                        S = sp.tile([P, nh * P], F16, tag="S")
                        nc.vector.tensor_scalar(
                            out=S[:],
                            in0=cst[:, 0 : nh * P],
                            scalar1=slot_sb[:, bi : bi + 1],
                            scalar2=ndst_sb[:, bi : bi + 1],
                            op0=mybir.AluOpType.is_equal,
                            op1=mybir.AluOpType.mult,
                        )
                        for h in range(nh):
                            ws = lo + h
                            nc.tensor.matmul(
                                ps[:, ws * P : (ws + 1) * P],
                                lhsT=gt[:, k * OUT : (k + 1) * OUT],
                                rhs=S[:, h * P : (h + 1) * P],
                                start=(first_mm.get((ownf, g, ws // 4)) == (bi, h)),
                                stop=(last_mm.get((ownf, g, ws // 4)) == (bi, h)),
                            )

                # ---- own-shard edges: gather from local h_mine, overlapped
                # with the AllGather ----
                for g, (gw0, nw) in enumerate(groups):
                    L = int(chunk_len[g][4])
                    if L == 0:
                        nc.vector.memset(ownAgg[:, gw0 * P : (gw0 + nw) * P], 0)
                        continue
                    psO = gpsum.tile([P, (WPG + 1) * P], F32, space="PSUM")
                    gt = gp.tile([P, (L // 128) * OUT], F16, tag="G")
                    o16 = int(chunk_off[g][4]) // 16
                    nc.gpsimd.dma_gather(
                        out_ap=gt[:].rearrange("p (b f) -> p b f", b=L // 128),
                        in_ap=h_mine[0 : cfg.Np, :],
                        idxs_ap=idx_sb[:, o16 : o16 + L // 16],
                        num_idxs=L,
                        num_idxs_reg=L,
                        elem_size=OUT,
                        single_packet=False,
                    )
                    do_blocks(g, 4, gt, psO, True)
                    nc.vector.tensor_copy(
                        out=ownAgg[:, gw0 * P : (gw0 + nw) * P], in_=psO[:, 0 : nw * P]
                    )

                for g, (gw0, nw) in enumerate(groups):
                    psg = gpsum.tile([P, (WPG + 1) * P], F32, space="PSUM")
                    gts = []
                    for r in range(4):
                        L = int(chunk_len[g][r])
                        if L == 0:
                            gts.append(None)
                            continue
                        gt = gp.tile([P, (L // 128) * OUT], F16, tag="G")
                        o16 = int(chunk_off[g][r]) // 16
                        if os.environ.get("GNN_NO_GATHER"):
                            nc.vector.memset(gt[:], 0)
                        else:
                            nc.gpsimd.dma_gather(
                                out_ap=gt[:].rearrange("p (b f) -> p b f", b=L // 128),
                                in_ap=h_all[bases[r] : bases[r] + rlen, :],
                                idxs_ap=idx_sb[:, o16 : o16 + L // 16],
                                num_idxs=L,
                                num_idxs_reg=L,
                                elem_size=OUT,
                                single_packet=False,
                            )
                        gts.append(gt)
                    # blocks of this group, in emission order (r, k)
                    for r in range(4):
                        if gts[r] is not None:
                            do_blocks(g, r, gts[r], psg, False)
                    # drain aggT (psum f32 -> sbuf f16), then add own partials
                    aggT = agp.tile([P, nw * P], F16)
                    nc.vector.tensor_copy(out=aggT[:], in_=psg[:, 0 : nw * P])
                    nc.vector.tensor_tensor(
                        out=aggT[:],
                        in0=aggT[:],
                        in1=ownAgg[:, gw0 * P : (gw0 + nw) * P],
                        op=mybir.AluOpType.add,
                    )
                    # up-projection per window + bias, drain on Act engine
                    ost = osb.tile([P, nw * IN], F16)
                    for wi in range(nw):
                        ps2 = upsum.tile([P, IN], F32, space="PSUM")
                        for lo2 in range(0, IN, 512):
                            hi2 = min(lo2 + 512, IN)
                            nc.tensor.matmul(
                                ps2[:, lo2:hi2],
                                lhsT=aggT[:, wi * P : (wi + 1) * P],
                                rhs=wgu_sb[:, lo2:hi2],
                                start=True,
                                stop=False,
                            )
                            nc.tensor.matmul(
                                ps2[:, lo2:hi2],
                                lhsT=cst[0:1, ONES_O : ONES_O + P],
                                rhs=cst[0:1, BU2_O + lo2 : BU2_O + hi2],
                                start=False,
                                stop=True,
                            )
                        nc.scalar.activation(
                            out=ost[:, wi * IN : (wi + 1) * IN],
                            in_=ps2[:],
                            func=mybir.ActivationFunctionType.Copy,
                        )
                    nc.sync.dma_start(
                        out=out[gw0 * P : (gw0 + nw) * P, :].rearrange(
                            "(w p) f -> p w f", p=P
                        ),
                        in_=ost[:].rearrange("p (w f) -> p w f", w=nw),
                    )

    nc.compile()
    return nc


_GRAPH_CACHE = {}


def kernel(features, Wd, bd, Wg, bg, Wu, bu, src, dst):
    features = np.asarray(features)
    N, IN = features.shape
    OUT = np.asarray(Wd).shape[1]
    E = np.asarray(src).shape[0]
    cfg = Cfg(N, E, IN, OUT)

    in_maps, node_core, node_slot, prof = host_prep(
        cfg, features, Wd, bd, Wg, bg, Wu, bu, src, dst
    )
    key = (N, E, IN, OUT, prof["T"], prof["blocks"])
    nc = _GRAPH_CACHE.get(key)
    if nc is None:
        nc = build_graph(cfg, prof)
        _GRAPH_CACHE[key] = nc

    res = run_bass_kernel_spmd(nc, in_maps, core_ids=list(range(cfg.C)))
    allo = np.stack([np.asarray(res.results[i]["out"]) for i in range(cfg.C)])
    return allo[node_core, node_slot, :].astype(np.float32)


# revision 16
# speedup vs baseline: 1.0002x; 1.0002x over previous
"""AdapterGNN distributed Trainium2 kernel (8 NeuronCores, Bass/Tile).

out = norm_dst * segsum_dst( ((X*norm_src) @ Wd + norm_src*bd)[src] ) @ (Wg@Wu) + (bg@Wu+bu)

Sharding: nodes split across 8 cores; edges partitioned by dst owner. The
down-projected node features h (fp16) are AllGathered into a DRAM table so
per-edge h[src] reads are core-local.

Aggregation: edges are gathered DENSELY (no per-dst rectangular padding) with
batched dma_gather instructions — one per (window-group, table-region) — into
an SBUF stream G laid out [edge%128 (partition), edge//128 (block), feat].
The int16 gather index limits one instruction to a 32704-row table region, so
the table is covered by 4 overlapping regions and each edge is routed to the
region (= table quarter) containing its src row. The segment-sum is computed
on the PE: for each 128-edge block, a selection matrix S[e, d] =
is_equal(iota, slot_e) * norm_dst_e is built by one DVE tensor_scalar op and
psum[f, d] += G_blk^T-contract-S accumulates aggT per window directly (no
transpose needed). Per-(window, region) edge counts are shared across cores
(max + pad with slot=-1 sentinels) so the block -> psum-slice mapping is
identical on every core. The up-projection uses the fused (Wg@Wu) weight; the
bias rides a K=1 matmul; psum drains to fp16 on the Activation engine.

Self-contained: requires only numpy + concourse (+ TRN2 cores via axon).
"""

import os

import numpy as np

import concourse.bacc as bacc
import concourse.mybir as mybir
import concourse.tile as tile
from concourse.bass_utils import run_bass_kernel_spmd

F32 = mybir.dt.float32
F16 = mybir.dt.float16
I16 = mybir.dt.int16

P = 128  # partitions

RLEN = 32704          # rows addressable by one int16-indexed gather
WPG = 6               # windows per group (psum budget: (WPG+1)*128 f32 = 2 banks)
MAXH = 4              # max window-halves one block may span
IOTA_W = MAXH * P     # iota width in consts
ONES_O = IOTA_W       # ones column block offset in consts
BU2_O = IOTA_W + P    # bias row offset in consts


class Cfg:
    def __init__(self, n_nodes, n_edges, in_dim, out_dim, n_cores=8):
        self.N = n_nodes
        self.E = n_edges
        self.IN = in_dim          # 768
        self.OUT = out_dim        # 128 (must be 128)
        self.C = n_cores
        assert out_dim == P
        self.NpReal = (n_nodes + n_cores - 1) // n_cores   # real nodes per core
        self.W = (self.NpReal + P - 1) // P                # windows per core
        self.Np = self.W * P                               # padded nodes/core
        self.KC = in_dim // P                              # full K chunks (6)
        assert in_dim % P == 0
        self.KIN = self.KC + 1                             # +1 chunk for (norm,bias) row
        self.n_rows = n_cores * self.Np


def _ceil128(x):
    return int(-(-int(x) // 128) * 128)


def host_prep(cfg, features, Wd, bd, Wg, bg, Wu, bu, src, dst):
    """Returns (in_maps, node_core, node_slot, prof)."""
    C, N, Np, W = cfg.C, cfg.N, cfg.Np, cfg.W
    src = np.asarray(src).astype(np.int64)
    dst = np.asarray(dst).astype(np.int64)
    features = np.asarray(features, dtype=np.float32)

    out_deg = np.bincount(src, minlength=N)
    in_deg = np.bincount(dst, minlength=N)
    norm_src = 1.0 / np.sqrt(np.maximum(out_deg, 1.0))
    norm_dst = 1.0 / np.sqrt(np.maximum(in_deg, 1.0))

    # node -> core; slot = in-degree order (keeps per-window edge counts
    # similar across cores -> tight shared maxima). The h table row equals the
    # slot the down-projection writes to, so table_row is slot-keyed.
    node_core = np.minimum(np.arange(N) // cfg.NpReal, C - 1)
    node_slot = np.empty(N, dtype=np.int64)
    for c in range(C):
        ns = np.where(node_core == c)[0]
        order = np.argsort(-in_deg[ns], kind="stable")
        node_slot[ns[order]] = np.arange(len(ns))
    table_row = node_core * Np + node_slot

    # regions 0-3: overlapping 32704-row windows of the REMOTE table (edges
    # whose src lives on another core), flex-balanced between adjacent regions
    # in the overlap zones so per-(window, region) counts are near-equal.
    # region 4: OWN edges (src on dst's core), gathered from local h_mine
    # (idx = local slot) concurrently with the AllGather.
    n_rows = cfg.n_rows
    rlen = min(RLEN, n_rows)
    nbase = n_rows - rlen
    bases = [0, nbase // 3, (2 * nbase) // 3, nbase]

    erow = table_row[src]
    ecore = node_core[dst]
    eslot = node_slot[dst]
    ew = eslot // P
    ep = eslot % P
    own = node_core[src] == ecore

    # region whose base is the highest <= erow; flexible iff also inside the
    # previous region's window
    zb = np.searchsorted(np.array(bases[1:]), erow, side="right")
    basesa = np.array(bases)
    flex = (zb >= 1) & (erow < basesa[np.maximum(zb - 1, 0)] + rlen) & ~own
    key = ecore * W + ew

    fcnt = np.zeros((C * W, 4), dtype=np.int64)
    np.add.at(fcnt, (key[~own & ~flex], zb[~own & ~flex]), 1)
    zcnt = np.zeros((C * W, 3), dtype=np.int64)
    np.add.at(zcnt, (key[flex], zb[flex] - 1), 1)

    tgt = (fcnt.sum(1) + zcnt.sum(1) + 3) // 4
    a = np.zeros((C * W, 3), dtype=np.int64)
    run = fcnt[:, 0]
    a[:, 0] = np.clip(tgt - run, 0, zcnt[:, 0])
    run = fcnt[:, 1] + (zcnt[:, 0] - a[:, 0])
    a[:, 1] = np.clip(tgt - run, 0, zcnt[:, 1])
    run = fcnt[:, 2] + (zcnt[:, 1] - a[:, 1])
    a[:, 2] = np.clip(tgt - run, 0, zcnt[:, 2])

    # per-edge final region
    ereg = np.where(own, 4, zb)
    fi = np.where(flex)[0]
    fz = zb[fi] - 1
    order = np.lexsort((erow[fi], fz, key[fi]))
    fi, fz = fi[order], fz[order]
    segid = key[fi] * 3 + fz
    uniq, counts = np.unique(segid, return_counts=True)
    cum = np.concatenate([[0], np.cumsum(counts)])
    frank = np.arange(len(fi)) - cum[np.searchsorted(uniq, segid)]
    ereg[fi] = fz + (frank >= a[key[fi], fz])

    # per-(core, window, region) counts and shared maxima
    NC = np.zeros((C, W, 5), dtype=np.int64)
    np.add.at(NC, (ecore, ew, ereg), 1)
    M = NC.max(axis=0)  # [W, 5]

    # groups of windows
    groups = []
    w0 = 0
    while w0 < W:
        nw = min(WPG, W - w0)
        groups.append((w0, nw))
        w0 += nw
    NG = len(groups)

    # chunk layout: all OWN chunks first (they run during the AllGather),
    # then remote chunks per (group, region)
    chunk_off = np.zeros((NG, 5), dtype=np.int64)
    chunk_len = np.zeros((NG, 5), dtype=np.int64)
    blocks = []  # (g, r, k_in_chunk, lo_seg, n_halves, bidx_global)
    off = 0

    def add_chunk(g, r, gw0, nw):
        nonlocal off
        seg = M[gw0 : gw0 + nw, r]
        L = _ceil128(seg.sum())
        chunk_off[g, r] = off
        chunk_len[g, r] = L
        bcum = np.concatenate([[0], np.cumsum(seg)])
        for k in range(L // 128):
            p0, p1 = k * 128, k * 128 + 127
            lo = int(np.searchsorted(bcum[1:], p0, side="right"))
            hi = int(np.searchsorted(bcum[1:], p1, side="right"))
            lo, hi = min(lo, nw), min(hi, nw)
            nh = hi - lo + 1
            assert nh <= MAXH, f"block spans {nh} windows"
            blocks.append([g, r, k, lo, nh, len(blocks)])
        off += L

    for g, (gw0, nw) in enumerate(groups):
        add_chunk(g, 4, gw0, nw)
    for g, (gw0, nw) in enumerate(groups):
        for r in range(4):
            add_chunk(g, r, gw0, nw)
    T = int(off)            # total stream length
    NBLK = len(blocks)

    # start/stop flags per (own?, g, psum zero-region): a 2KB PSUM zero region
    # holds 4 window slices (512B each). start=True marks the WHOLE region
    # pending-zero (lazy zeroing), so exactly one start and one stop per
    # region, on its chronologically first/last matmul. Own chunks accumulate
    # into their own psum tiles, so they get a separate flag namespace.
    first_mm = {}
    last_mm = {}
    for b in blocks:
        g, r, k, lo, nh, bi = b
        for h in range(nh):
            key2 = (r == 4, g, (lo + h) // 4)
            if key2 not in first_mm:
                first_mm[key2] = (bi, h)
            last_mm[key2] = (bi, h)

    # shared profile for build_graph (hashable)
    prof = {
        "groups": tuple(groups),
        "chunk_off": tuple(map(tuple, chunk_off)),
        "chunk_len": tuple(map(tuple, chunk_len)),
        "blocks": tuple(tuple(b) for b in blocks),
        "first": frozenset((k, v) for k, v in first_mm.items()),
        "last": frozenset((k, v) for k, v in last_mm.items()),
        "T": T,
        "NBLK": NBLK,
        "bases": tuple(bases),
        "rlen": rlen,
    }

    # fused weights
    Wgu = (np.asarray(Wg, np.float64) @ np.asarray(Wu, np.float64)).astype(np.float32)
    bu2 = (np.asarray(bg, np.float64) @ np.asarray(Wu, np.float64) + bu).astype(np.float32)

    wd_h = np.zeros((P, cfg.KIN * cfg.OUT), dtype=np.float16)
    for cc in range(cfg.KC):
        wd_h[:, cc * cfg.OUT : (cc + 1) * cfg.OUT] = Wd[cc * P : (cc + 1) * P, :]
    wd_h[0, cfg.KC * cfg.OUT : (cfg.KC + 1) * cfg.OUT] = bd
    wgu_h = Wgu.astype(np.float16)

    consts = np.zeros((P, BU2_O + cfg.IN), dtype=np.float16)
    consts[:, 0:IOTA_W] = np.arange(IOTA_W, dtype=np.float16)[None, :]
    consts[:, ONES_O : ONES_O + P] = 1.0
    consts[:, BU2_O : BU2_O + cfg.IN] = bu2.astype(np.float16)[None, :]

    # per-core streams
    S16 = T // 16
    in_maps = []
    # intra-chunk window segment offsets (shared)
    segoff = np.zeros((W, 5), dtype=np.int64)
    for g, (gw0, nw) in enumerate(groups):
        for r in range(5):
            segoff[gw0 : gw0 + nw, r] = chunk_off[g, r] + np.concatenate(
                [[0], np.cumsum(M[gw0 : gw0 + nw, r])[:-1]]
            )

    blk_lo = np.zeros(T // 128, dtype=np.int64)
    for g, r, k, lo, nh, bi in blocks:
        assert chunk_off[g, r] // 128 + k == bi
        blk_lo[bi] = lo

    eslot_src = node_slot[src]  # own-region gather row (local h_mine slot)
    gw0_of = np.array([groups[gg][0] for gg in range(NG)])

    for c in range(C):
        em = np.where(ecore == c)[0]
        ewc, erc, epc = ew[em], ereg[em], ep[em]
        order = np.lexsort((epc, erc, ewc))
        em, ewc, erc, epc = em[order], ewc[order], erc[order], epc[order]
        segid = ewc * 5 + erc
        uniq, counts = np.unique(segid, return_counts=True)
        cum = np.concatenate([[0], np.cumsum(counts)])
        rank = np.arange(len(em)) - cum[np.searchsorted(uniq, segid)]
        pos = segoff[ewc, erc] + rank
        assert (rank < M[ewc, erc]).all()

        idx_s = np.zeros(T, dtype=np.int64)
        slotg = np.full(T, -1, dtype=np.int64)   # slot within group (wi*128+p)
        ndst_s = np.zeros(T, dtype=np.float32)
        idxval = np.where(erc == 4, eslot_src[em], erow[em] - basesa[np.minimum(erc, 3)])
        idx_s[pos] = idxval
        g_of_w = np.minimum(ewc // WPG, NG - 1)
        wi = ewc - gw0_of[g_of_w]
        slotg[pos] = wi * P + epc
        ndst_s[pos] = norm_dst[dst[em]].astype(np.float32)
        assert (idx_s >= 0).all() and (idx_s < max(rlen, Np)).all()

        # per-block relative slots
        slot_rel = slotg.reshape(-1, 128) - blk_lo[:, None] * P
        slot_rel[slotg.reshape(-1, 128) < 0] = -1
        assert (slot_rel < MAXH * P).all()

        idx16 = np.zeros((P, S16), dtype=np.int16)
        sidx = np.arange(S16) * 16
        for p in range(P):
            idx16[p, :] = idx_s[sidx + (p % 16)]
        slotv = np.ascontiguousarray(slot_rel.T.astype(np.float32))   # [128, NBLK]
        ndstv = np.ascontiguousarray(
            ndst_s.reshape(-1, 128).T.astype(np.float32)
        )

        # xa: window-blocked [p, w*KIN*128 + cc*128 + n]
        nt_ids = np.where(node_core == np.int64(c))[0]
        n_real = len(nt_ids)
        xs = (features[nt_ids, :] * norm_src[nt_ids, None]).astype(np.float16)
        xa = np.zeros((P, W * cfg.KIN * P), dtype=np.float16)
        xs_slot = np.zeros((Np, cfg.IN), dtype=np.float16)
        xs_slot[node_slot[nt_ids], :] = xs
        nsr = np.zeros(Np, dtype=np.float16)
        nsr[node_slot[nt_ids]] = norm_src[nt_ids].astype(np.float16)
        for w in range(W):
            blkb = w * cfg.KIN * P
            rows = xs_slot[w * P : (w + 1) * P, :]  # [128 nodes, IN]
            for cc in range(cfg.KC):
                xa[:, blkb + cc * P : blkb + (cc + 1) * P] = rows[:, cc * P : (cc + 1) * P].T
            xa[0, blkb + cfg.KC * P : blkb + (cfg.KC + 1) * P] = nsr[w * P : (w + 1) * P]

        in_maps.append(
            {
                "xa": xa,
                "idx": idx16,
                "slotv": slotv,
                "ndstv": ndstv,
                "wd": wd_h,
                "wgu": wgu_h,
                "consts": consts,
            }
        )

    return in_maps, node_core, node_slot, prof


def build_graph(cfg, prof):
    """Build the SPMD Bass graph (same for all cores)."""
    W = cfg.W
    OUT, IN = cfg.OUT, cfg.IN
    groups = prof["groups"]
    chunk_off = prof["chunk_off"]
    chunk_len = prof["chunk_len"]
    blocks = prof["blocks"]
    first_mm = dict(prof["first"])
    last_mm = dict(prof["last"])
    T = prof["T"]
    NBLK = prof["NBLK"]
    bases = prof["bases"]
    rlen = prof["rlen"]
    S16 = T // 16

    nc = bacc.Bacc(None, target_bir_lowering=False)
    xa = nc.declare_dram_parameter("xa", [P, W * cfg.KIN * P], F16, False)
    idx = nc.declare_dram_parameter("idx", [P, S16], I16, False)
    slotv = nc.declare_dram_parameter("slotv", [P, NBLK], F32, False)
    ndstv = nc.declare_dram_parameter("ndstv", [P, NBLK], F32, False)
    wd = nc.declare_dram_parameter("wd", [P, cfg.KIN * OUT], F16, False)
    wgu = nc.declare_dram_parameter("wgu", [OUT, IN], F16, False)
    consts = nc.declare_dram_parameter("consts", [P, BU2_O + IN], F16, False)
    out = nc.declare_dram_parameter("out", [cfg.Np, IN], F16, True)

    with tile.TileContext(nc) as tc:
        with (
            tc.tile_pool(name="dram", bufs=1, space="DRAM") as dram,
            tc.tile_pool(name="gpsum", bufs=2, space="PSUM") as gpsum,
            tc.tile_pool(name="upsum", bufs=2, space="PSUM") as upsum,
        ):
            h_mine = dram.tile([cfg.Np, OUT], F16)
            h_all = dram.tile([cfg.n_rows, OUT], F16)

            # ---- phase A: down-projection ----
            with (
                tc.tile_pool(name="aconst", bufs=1) as aconst,
                tc.tile_pool(name="xat", bufs=2) as xap,
                tc.tile_pool(name="hst", bufs=1) as hstp,
            ):
                wd_sb = aconst.tile([P, cfg.KIN * OUT], F16)
                nc.sync.dma_start(out=wd_sb[:], in_=wd[:, :])
                h_stage = hstp.tile([P, W * OUT], F16)
                xa_v = xa[:, :].rearrange("p (w x) -> p w x", w=W)
                NQ = 4
                wq = (W + NQ - 1) // NQ
                for qd in range(NQ):
                    lo = qd * wq
                    hi = min(lo + wq, W)
                    xt = xap.tile([P, (hi - lo) * cfg.KIN * P], F16)
                    nc.sync.dma_start(
                        out=xt[:].rearrange("p (w x) -> p w x", w=hi - lo),
                        in_=xa_v[:, lo:hi, :],
                    )
                    for w in range(lo, hi):
                        blkb = (w - lo) * cfg.KIN * P
                        psfull = gpsum.tile([P, (WPG + 1) * P], F32, space="PSUM", tag="PS")
                        ps = psfull[:, 0:OUT]
                        for ccx in range(cfg.KC):
                            nc.tensor.matmul(
                                ps[:],
                                lhsT=xt[:, blkb + ccx * P : blkb + (ccx + 1) * P],
                                rhs=wd_sb[:, ccx * OUT : (ccx + 1) * OUT],
                                start=(ccx == 0),
                                stop=False,
                            )
                        nc.tensor.matmul(
                            ps[:],
                            lhsT=xt[0:1, blkb + cfg.KC * P : blkb + (cfg.KC + 1) * P],
                            rhs=wd_sb[0:1, cfg.KC * OUT : cfg.KC * OUT + OUT],
                            start=False,
                            stop=True,
                        )
                        nc.vector.tensor_copy(
                            out=h_stage[:, w * OUT : (w + 1) * OUT], in_=ps[:]
                        )
                # one DMA: h_stage [p, w, f] -> h_mine rows (w*128+p)
                nc.sync.dma_start(
                    out=h_mine[:, :].rearrange("(w p) f -> p w f", p=P),
                    in_=h_stage[:].rearrange("p (w f) -> p w f", w=W),
                )

            # ---- all-gather h ----
            if os.environ.get("GNN_NO_COLL"):
                # debug: fake the collective with a local copy (wrong numbers)
                for c in range(cfg.C):
                    nc.sync.dma_start(
                        out=h_all[c * cfg.Np : (c + 1) * cfg.Np, :], in_=h_mine[:, :]
                    )
            else:
                nc.gpsimd.collective_compute(
                    "AllGather",
                    mybir.AluOpType.bypass,
                    replica_groups=[list(range(cfg.C))],
                    ins=[h_mine[:].opt()],
                    outs=[h_all[:].opt()],
                )

            # ---- phase B: edge aggregation + up-projection ----
            with (
                tc.tile_pool(name="bconst", bufs=1) as bconst,
                tc.tile_pool(name="gp", bufs=3) as gp,
                tc.tile_pool(name="sp", bufs=8) as sp,
                tc.tile_pool(name="agp", bufs=2) as agp,
                tc.tile_pool(name="osb", bufs=2) as osb,
            ):
                wgu_sb = bconst.tile([OUT, IN], F16)
                nc.sync.dma_start(out=wgu_sb[:], in_=wgu[:, :])
                cst = bconst.tile([P, BU2_O + IN], F16)
                nc.sync.dma_start(out=cst[:], in_=consts[:, :])
                idx_sb = bconst.tile([P, S16], I16)
                nc.sync.dma_start(out=idx_sb[:], in_=idx[:, :])
                slot_sb = bconst.tile([P, NBLK], F32)
                nc.sync.dma_start(out=slot_sb[:], in_=slotv[:, :])
                ndst_sb = bconst.tile([P, NBLK], F32)
                nc.sync.dma_start(out=ndst_sb[:], in_=ndstv[:, :])
                ownAgg = bconst.tile([P, W * P], F16)

                def do_blocks(g, r, gt, ps, ownf):
                    for g2, r2, k, lo, nh, bi in blocks:
                        if g2 != g or r2 != r:
                            continue
                        S = sp.tile([P, nh * P], F16, tag="S")
                        nc.vector.tensor_scalar(
                            out=S[:],
                            in0=cst[:, 0 : nh * P],
                            scalar1=slot_sb[:, bi : bi + 1],
                            scalar2=ndst_sb[:, bi : bi + 1],
                            op0=mybir.AluOpType.is_equal,
                            op1=mybir.AluOpType.mult,
                        )
                        for h in range(nh):
                            ws = lo + h
                            nc.tensor.matmul(
                                ps[:, ws * P : (ws + 1) * P],
                                lhsT=gt[:, k * OUT : (k + 1) * OUT],
                                rhs=S[:, h * P : (h + 1) * P],
                                start=(first_mm.get((ownf, g, ws // 4)) == (bi, h)),
                                stop=(last_mm.get((ownf, g, ws // 4)) == (bi, h)),
                            )

                # ---- own-shard edges: gather from local h_mine, overlapped
                # with the AllGather ----
                for g, (gw0, nw) in enumerate(groups):
                    L = int(chunk_len[g][4])
                    if L == 0:
                        nc.vector.memset(ownAgg[:, gw0 * P : (gw0 + nw) * P], 0)
                        continue
                    psO = gpsum.tile([P, (WPG + 1) * P], F32, space="PSUM", tag="PS")
                    gt = gp.tile([P, (L // 128) * OUT], F16, tag="G")
                    o16 = int(chunk_off[g][4]) // 16
                    nc.gpsimd.dma_gather(
                        out_ap=gt[:].rearrange("p (b f) -> p b f", b=L // 128),
                        in_ap=h_mine[0 : cfg.Np, :],
                        idxs_ap=idx_sb[:, o16 : o16 + L // 16],
                        num_idxs=L,
                        num_idxs_reg=L,
                        elem_size=OUT,
                        single_packet=False,
                    )
                    do_blocks(g, 4, gt, psO, True)
                    nc.vector.tensor_copy(
                        out=ownAgg[:, gw0 * P : (gw0 + nw) * P], in_=psO[:, 0 : nw * P]
                    )

                for g, (gw0, nw) in enumerate(groups):
                    psg = gpsum.tile([P, (WPG + 1) * P], F32, space="PSUM", tag="PS")
                    gts = []
                    for r in range(4):
                        L = int(chunk_len[g][r])
                        if L == 0:
                            gts.append(None)
                            continue
                        gt = gp.tile([P, (L // 128) * OUT], F16, tag="G")
                        o16 = int(chunk_off[g][r]) // 16
                        if os.environ.get("GNN_NO_GATHER"):
                            nc.vector.memset(gt[:], 0)
                        else:
                            nc.gpsimd.dma_gather(
                                out_ap=gt[:].rearrange("p (b f) -> p b f", b=L // 128),
                                in_ap=h_all[bases[r] : bases[r] + rlen, :],
                                idxs_ap=idx_sb[:, o16 : o16 + L // 16],
                                num_idxs=L,
                                num_idxs_reg=L,
                                elem_size=OUT,
                                single_packet=False,
                            )
                        gts.append(gt)
                    # blocks of this group, in emission order (r, k)
                    for r in range(4):
                        if gts[r] is not None:
                            do_blocks(g, r, gts[r], psg, False)
                    # drain aggT (psum f32 -> sbuf f16), then add own partials
                    aggT = agp.tile([P, nw * P], F16)
                    nc.vector.tensor_copy(out=aggT[:], in_=psg[:, 0 : nw * P])
                    nc.vector.tensor_tensor(
                        out=aggT[:],
                        in0=aggT[:],
                        in1=ownAgg[:, gw0 * P : (gw0 + nw) * P],
                        op=mybir.AluOpType.add,
                    )
                    # up-projection per window + bias, drain on Act engine
                    ost = osb.tile([P, nw * IN], F16)
                    for wi in range(nw):
                        ps2 = upsum.tile([P, IN], F32, space="PSUM", tag="UP")
                        for lo2 in range(0, IN, 512):
                            hi2 = min(lo2 + 512, IN)
                            nc.tensor.matmul(
                                ps2[:, lo2:hi2],
                                lhsT=aggT[:, wi * P : (wi + 1) * P],
                                rhs=wgu_sb[:, lo2:hi2],
                                start=True,
                                stop=False,
                            )
                            nc.tensor.matmul(
                                ps2[:, lo2:hi2],
                                lhsT=cst[0:1, ONES_O : ONES_O + P],
                                rhs=cst[0:1, BU2_O + lo2 : BU2_O + hi2],
                                start=False,
                                stop=True,
                            )
                        nc.scalar.activation(
                            out=ost[:, wi * IN : (wi + 1) * IN],
                            in_=ps2[:],
                            func=mybir.ActivationFunctionType.Copy,
                        )
                    nc.sync.dma_start(
                        out=out[gw0 * P : (gw0 + nw) * P, :].rearrange(
                            "(w p) f -> p w f", p=P
                        ),
                        in_=ost[:].rearrange("p (w f) -> p w f", w=nw),
                    )

    nc.compile()
    return nc


_GRAPH_CACHE = {}


def kernel(features, Wd, bd, Wg, bg, Wu, bu, src, dst):
    features = np.asarray(features)
    N, IN = features.shape
    OUT = np.asarray(Wd).shape[1]
    E = np.asarray(src).shape[0]
    cfg = Cfg(N, E, IN, OUT)

    in_maps, node_core, node_slot, prof = host_prep(
        cfg, features, Wd, bd, Wg, bg, Wu, bu, src, dst
    )
    key = (N, E, IN, OUT, prof["T"], prof["blocks"])
    nc = _GRAPH_CACHE.get(key)
    if nc is None:
        nc = build_graph(cfg, prof)
        _GRAPH_CACHE[key] = nc

    res = run_bass_kernel_spmd(nc, in_maps, core_ids=list(range(cfg.C)))
    allo = np.stack([np.asarray(res.results[i]["out"]) for i in range(cfg.C)])
    return allo[node_core, node_slot, :].astype(np.float32)
